# revision 23
# baseline (speedup 1.0000x reference)
"""Trainium2 Bass kernel for 16-head RoPE self-attention (S=2048, B=2, D=2048).

Sharding: 8 cores = 2 batches x 4 head-groups (4 heads each). Each core
computes qkv projection for its batch/heads, full attention over its 4
heads, and a partial output projection (its 4-head slice of Wo rows).
Host sums the 4 partial outputs per batch. No cross-core collectives.

Changes over the 404us baseline (measured ~382us):
  - startup: input DMAs staged in ascending-k slices split across the
    sync AND scalar queues (the only two HWDGE-capable issuers); big
    trailing loads stay on sync so ring-credit blocking never delays
    the scalar queue's copies. pmat is loaded first — the rope
    rot-matmul blocks the in-order PE queue if it lands late.
  - phase 1: lag-1 rope emission (proj p's rope emitted after proj
    p+1's matmuls) so the rot matmul never heads the PE queue while
    its raw copy is in flight.
  - phase 2: output-projection work is emitted in per-et units and
    interleaved into stripe 1's attention windows (4 per head), which
    are otherwise ScalarE(EXP)-gated; stripe 1's own units drain at
    the end (they need all heads' attention).
  - softmax denominator as RUNNING partial sums: exps 0..11 through a
    small DVE tree while later jts stream, exps 12..15 folded in one
    short add each right after their EXP, so the total is ready
    ~0.6us after the last EXP and only 2 ones-matmuls per head are
    needed (was 8). A leaf-pairwise tree needs ~3.4us of post-EXP
    latency, which held the scores-PSUM rotation slot, stalled the
    next head's scores, and re-throttled HAM to half clock.
  - outputs are written bf16 (halves output DMA; host accumulates the
    4 partials in f32), one 2KB-run DMA per 128-row block/stripe.
"""

import os
import numpy as np
import ml_dtypes

S, B, D = 2048, 2, 2048
N_HEADS, DQK = 16, 128
ROPE_THETA = 500000.0
N_CORES = 8
CORES_PER_BATCH = 4
NH_LOC = N_HEADS // CORES_PER_BATCH  # 4 heads per core

LAST_RESULT = None  # BassKernelResults of the most recent run (for test.py)

_NC_CACHE = {}


def _build_nc(s, dmodel, nh_loc, d=DQK, sc=512):
    import concourse.tile as tile
    from concourse import bacc, mybir

    bf16 = mybir.dt.bfloat16
    f32 = mybir.dt.float32
    nk = dmodel // 128      # contraction tiles for the projections
    ns = s // 128           # sequence tiles (key side)
    nch = s // sc           # sequence chunks (query side / moving dim)
    hd = nh_loc * d         # local head-dim total (512)
    ne = dmodel // 128      # output-embedding tiles

    nc = bacc.Bacc("TRN2", target_bir_lowering=False, debug=False)
    xTq = nc.dram_tensor("xTq", [nch, 128, nk, sc], bf16, kind="ExternalInput")
    wq = nc.dram_tensor("wq", [128, nk, hd], bf16, kind="ExternalInput")
    wk = nc.dram_tensor("wk", [128, nk, hd], bf16, kind="ExternalInput")
    wv = nc.dram_tensor("wv", [128, nk, hd], bf16, kind="ExternalInput")
    wo = nc.dram_tensor("wo", [128, hd // 128, dmodel], bf16,
                        kind="ExternalInput")
    cosT = nc.dram_tensor("cosT", [d, s], bf16, kind="ExternalInput")
    sinT = nc.dram_tensor("sinT", [d, s], bf16, kind="ExternalInput")
    maskb = nc.dram_tensor("maskb", [128, ns], f32, kind="ExternalInput")
    pmat = nc.dram_tensor("pmat", [d, d], bf16, kind="ExternalInput")
    ones = nc.dram_tensor("ones", [128, 128], bf16, kind="ExternalInput")
    outT = nc.dram_tensor("outT", [dmodel, s], bf16, kind="ExternalOutput")

    outT_r = outT.rearrange("(e p) s -> p e s", p=128)

    with tile.TileContext(nc) as tc:
        with tc.tile_pool(name="const", bufs=1) as constp, \
             tc.tile_pool(name="store", bufs=1) as storep:
            qT_sb = storep.tile([128, nh_loc, s], bf16)
            kT_sb = storep.tile([128, nh_loc, s], bf16)
            v_sb = storep.tile([128, ns, hd], bf16)
            attn_sb = storep.tile([128, nh_loc, s], bf16)

            # ---- phase 1: qkv projection + rope -------------------------
            with tc.tile_pool(name="wqkv", bufs=1) as wp, \
                 tc.tile_pool(name="xch", bufs=2) as xp, \
                 tc.tile_pool(name="rope", bufs=3) as rp, \
                 tc.tile_pool(name="ps1", bufs=1, space="PSUM") as ps1, \
                 tc.tile_pool(name="ps1v", bufs=2, space="PSUM") as ps1v, \
                 tc.tile_pool(name="ps1r", bufs=2, space="PSUM") as ps1r:
                wq_sb = wp.tile([128, nk, hd], bf16, tag="wq")
                wk_sb = wp.tile([128, nk, hd], bf16, tag="wk")
                wv_sb = wp.tile([128, nk, hd], bf16, tag="wv")
                xc0 = xp.tile([128, nk, sc], bf16, tag="xc")
                # staged ascending-k loads. wq on the scalar queue, xc0 on
                # sync (the two HWDGE issuers) so issue costs parallelize.
                # Big trailing loads go on sync so ring-credit blocking
                # never delays the scalar queue, which must run the
                # raw/rot/v copies from ~14us on.
                # pmat first: it's tiny and the rope rot-matmul blocks the
                # in-order PE queue if it lands late.
                pmat_sb = constp.tile([128, d], bf16)
                nc.scalar.dma_start(pmat_sb[:], pmat[:])
                # first two k-tiles split into 4 column sub-DMAs each: a
                # single dma_start's packets land on only 1-2 DMA engines,
                # which at cold ~225ns/packet costs ~10us for the first
                # 128KB slice; 4 sub-issues spread the packets across
                # engines so the first matmul's inputs land ~4x sooner.
                for ks in (slice(0, 1), slice(1, 2)):
                    for c in range(4):
                        cs = slice(c * 128, (c + 1) * 128)
                        nc.scalar.dma_start(wq_sb[:, ks, cs], wq[:, ks, cs])
                        nc.sync.dma_start(xc0[:, ks, cs],
                                          xTq[0, :, ks, cs])
                g = 2
                for gq in (2, 4, 4, 4):
                    gs = slice(g, g + gq)
                    nc.scalar.dma_start(wq_sb[:, gs, :], wq[:, gs, :])
                    nc.sync.dma_start(xc0[:, gs, :], xTq[0, :, gs, :])
                    g += gq
                cos_sb = constp.tile([128, s], bf16)
                nc.scalar.dma_start(cos_sb[:], cosT[:])
                sin_sb = constp.tile([128, s], bf16)
                nc.scalar.dma_start(sin_sb[:], sinT[:])
                for gs in (slice(0, 4), slice(4, 8), slice(8, 12),
                           slice(12, 16)):
                    nc.sync.dma_start(wk_sb[:, gs, :], wk[:, gs, :])
                for gs in (slice(0, 4), slice(4, 8), slice(8, 12),
                           slice(12, 16)):
                    nc.sync.dma_start(wv_sb[:, gs, :], wv[:, gs, :])
                maskb_sb = constp.tile([128, ns], f32)
                nc.sync.dma_start(maskb_sb[:], maskb[:])
                ones_sb = constp.tile([128, 128], bf16)
                nc.sync.dma_start(ones_sb[:], ones[:])
                wo_sb = constp.tile([128, nh_loc, dmodel], bf16)
                nc.sync.dma_start(wo_sb[:], wo[:])

                def rope_combine(dstT, h, acc, csl, gi):
                    # q/k rope: raw -> (pmat matmul) rot; out = raw*cos+rot*sin
                    raw = rp.tile([128, sc], bf16, tag="raw", name=f"raw{gi}")
                    nc.scalar.copy(raw[:], acc[:])
                    rot_ps = ps1r.tile([128, sc], f32, tag="rot",
                                       name=f"rotp{gi}")
                    nc.tensor.matmul(rot_ps[:], pmat_sb[:], raw[:],
                                     start=True, stop=True)
                    rot = rp.tile([128, sc], bf16, tag="rot", name=f"rot{gi}")
                    nc.scalar.copy(rot[:], rot_ps[:])
                    t1 = rp.tile([128, sc], bf16, tag="t1", name=f"t1_{gi}")
                    nc.vector.tensor_mul(t1[:], raw[:], cos_sb[:, csl])
                    t2 = rp.tile([128, sc], bf16, tag="t2", name=f"t2_{gi}")
                    nc.vector.tensor_mul(t2[:], rot[:], sin_sb[:, csl])
                    nc.vector.tensor_add(dstT[:, h, csl], t1[:], t2[:])

                # lag-1 rope: emit proj p's rope AFTER proj p+1's matmuls so
                # the rot matmul never heads the PE queue while its raw
                # copy (ScalarE) is still in flight.
                pend_rope = [None]

                def queue_rope(*args):
                    if pend_rope[0] is not None:
                        pend_rope[0]()
                    pend_rope[0] = lambda: rope_combine(*args)

                def flush_rope():
                    if pend_rope[0] is not None:
                        pend_rope[0]()
                        pend_rope[0] = None

                gi = 0
                for ch in range(nch):
                    csl = slice(ch * sc, (ch + 1) * sc)
                    if ch == 0:
                        xc = xc0
                        # consume in DMA-arrival order: all q heads first
                        # (wq lands before wk)
                        proj_order = [("q", h) for h in range(nh_loc)] + \
                                     [("k", h) for h in range(nh_loc)]
                    else:
                        xc = xp.tile([128, nk, sc], bf16, tag="xc")
                        nc.sync.dma_start(xc[:], xTq[ch])
                        proj_order = [(t, h) for h in range(nh_loc)
                                      for t in ("q", "k")]
                    for t, h in proj_order:
                        hsl = slice(h * d, (h + 1) * d)
                        w_sb, dstT = ((wq_sb, qT_sb) if t == "q"
                                      else (wk_sb, kT_sb))
                        acc = ps1.tile([128, sc], f32, tag=f"acc{gi % 4}",
                                       name=f"acc{t}{h}c{ch}")
                        for k in range(nk):
                            nc.tensor.matmul(
                                acc[:], w_sb[:, k, hsl], xc[:, k, :],
                                start=(k == 0), stop=(k == nk - 1),
                            )
                        queue_rope(dstT, h, acc, csl, gi)
                        gi += 1
                    for stl in range(sc // 128):
                        st = ch * (sc // 128) + stl
                        ssl = slice(stl * 128, (stl + 1) * 128)
                        accv = ps1v.tile([128, hd], f32, tag="accv")
                        for k in range(nk):
                            nc.tensor.matmul(
                                accv[:], xc[:, k, ssl], wv_sb[:, k, :],
                                start=(k == 0), stop=(k == nk - 1),
                            )
                        if stl == 0:
                            flush_rope()
                        nc.scalar.copy(v_sb[:, st, :], accv[:])

            # ---- phase 2: attention + output projection -----------------
            sw = 2 * sc
            nst = s // sw
            with tc.tile_pool(name="expp", bufs=2) as expp, \
                 tc.tile_pool(name="ph2", bufs=3) as ph2p, \
                 tc.tile_pool(name="qd", bufs=5) as qdp, \
                 tc.tile_pool(name="outp", bufs=4) as outp, \
                 tc.tile_pool(name="ps2s", bufs=2, space="PSUM") as ps2s, \
                 tc.tile_pool(name="ps2p", bufs=2, space="PSUM") as ps2p, \
                 tc.tile_pool(name="ps2o", bufs=2, space="PSUM") as ps2o:

                def oproj_unit(ic, et, tail):
                    def emit():
                        esl = slice(et * 128, (et + 1) * 128)
                        ot = outp.tile([128, sw], bf16, tag="ot",
                                       name=f"ot{ic}_{et}")
                        for half in range(2):
                            osl = slice(ic * sw + half * sc,
                                        ic * sw + (half + 1) * sc)
                            op_ps = ps2o.tile([128, sc], f32, tag="oproj",
                                              name=f"op{ic}_{et}_{half}")
                            for ht in range(nh_loc):
                                nc.tensor.matmul(
                                    op_ps[:], wo_sb[:, ht, esl],
                                    attn_sb[:, ht, osl],
                                    start=(ht == 0), stop=(ht == nh_loc - 1),
                                )
                            dst = ot[:, half * sc:(half + 1) * sc]
                            # in the attention windows ScalarE is saturated
                            # with EXPs; in the tail it's idle, so split.
                            if tail and half == 0:
                                nc.scalar.copy(dst, op_ps[:])
                            else:
                                nc.vector.tensor_copy(dst, op_ps[:])
                        nc.sync.dma_start(
                            outT_r[:, et, ic * sw:(ic + 1) * sw], ot[:])
                    return emit

                pending = []
                for ic in range(nst):
                    for h in range(nh_loc):
                        hsl = slice(h * d, (h + 1) * d)
                        ex = expp.tile([128, ns, sw], bf16, tag="exp",
                                       name=f"ex{ic}_{h}")
                        # Denominator as RUNNING partial sums: exps 0..11 go
                        # through a small tree while later jts stream; exps
                        # 12..15 are folded in one short add each, emitted
                        # right after their EXP, so the total is ready
                        # ~0.6us after the last EXP (a leaf-pairwise tree
                        # needs ~3.4us of post-EXP latency, which stalled
                        # the next head's scores and re-throttled HAM).
                        quads = []
                        run = None
                        for jt in range(ns):
                            jsl = slice(jt * 128, (jt + 1) * 128)
                            sc_ps = ps2s.tile([128, sw], f32, tag="scores",
                                              name=f"scps{ic}_{h}_{jt}")
                            for half in range(2):
                                qsl = slice(ic * sw + half * sc,
                                            ic * sw + (half + 1) * sc)
                                nc.tensor.matmul(
                                    sc_ps[:, half * sc:(half + 1) * sc],
                                    kT_sb[:, h, jsl], qT_sb[:, h, qsl],
                                    start=True, stop=True)
                            nc.scalar.activation(
                                ex[:, jt, :], sc_ps[:],
                                mybir.ActivationFunctionType.Exp,
                                bias=maskb_sb[:, jt:jt + 1], scale=1.0,
                            )
                            if jt in (3, 7, 11):
                                a = jt // 4
                                p0 = ph2p.tile([128, sw], bf16, tag="pair")
                                nc.vector.tensor_add(p0[:], ex[:, 4 * a, :],
                                                     ex[:, 4 * a + 1, :])
                                p1 = ph2p.tile([128, sw], bf16, tag="pair2")
                                nc.vector.tensor_add(
                                    p1[:], ex[:, 4 * a + 2, :],
                                    ex[:, 4 * a + 3, :])
                                q4 = qdp.tile([128, sw], bf16, tag="quad")
                                nc.vector.tensor_add(q4[:], p0[:], p1[:])
                                quads.append(q4)
                                if a == 2:
                                    s01 = qdp.tile([128, sw], bf16,
                                                   tag="s01", bufs=2)
                                    nc.vector.tensor_add(s01[:], quads[0][:],
                                                         quads[1][:])
                                    run = qdp.tile([128, sw], bf16,
                                                   tag="run", bufs=2)
                                    nc.vector.tensor_add(run[:], s01[:],
                                                         quads[2][:])
                            elif jt >= 12:
                                nxt = qdp.tile([128, sw], bf16, tag="run",
                                               bufs=2,
                                               name=f"run{ic}_{h}_{jt}")
                                nc.vector.tensor_add(nxt[:], run[:],
                                                     ex[:, jt, :])
                                run = nxt
                            # fill the EXP-gated stream with oproj work from
                            # the previous stripe
                            if pending and jt in (3, 7, 11, 15):
                                pending.pop(0)()
                        # r lives in the oproj pool (idle in stripe 0,
                        # compatible rotation in stripe 1) so the scores
                        # rotation is never held by the r->recip chain —
                        # that stalled every next head's scores 2.6-4.9us
                        # and re-throttled HAM.
                        inv = ph2p.tile([128, sw], f32, tag="inv")
                        for half in range(2):
                            fsl = slice(half * sc, (half + 1) * sc)
                            r_ps = ps2o.tile([128, sc], f32, tag="oproj",
                                             name=f"rps{ic}_{h}_{half}")
                            nc.tensor.matmul(r_ps[:], ones_sb[:],
                                             run[:, fsl],
                                             start=True, stop=True)
                            nc.vector.reciprocal_approx_fast(inv[:, fsl],
                                                             r_ps[:])
                        pvs = []
                        for half in range(2):
                            fsl = slice(half * sc, (half + 1) * sc)
                            pv_ps = ps2p.tile([128, sc], f32, tag="pv")
                            for jt in range(ns):
                                nc.tensor.matmul(
                                    pv_ps[:], v_sb[:, jt, hsl],
                                    ex[:, jt, fsl],
                                    start=(jt == 0), stop=(jt == ns - 1),
                                )
                            pvs.append(pv_ps)
                        for half in range(2):
                            fsl = slice(half * sc, (half + 1) * sc)
                            asl = slice(ic * sw + half * sc,
                                        ic * sw + (half + 1) * sc)
                            nc.vector.tensor_mul(attn_sb[:, h, asl],
                                                 pvs[half][:], inv[:, fsl])
                    if ic == 0:
                        pending = [oproj_unit(0, et, False)
                                   for et in range(ne)]
                for f in pending:
                    f()
                for et in range(ne):
                    oproj_unit(nst - 1, et, True)()

    nc.compile()
    return nc


def _get_nc(s=S, dmodel=D, nh_loc=NH_LOC):
    key = (s, dmodel, nh_loc)
    if key not in _NC_CACHE:
        _NC_CACHE[key] = _build_nc(s, dmodel, nh_loc)
    return _NC_CACHE[key]


def _rope_tables(s, d, dtype=np.float32):
    inv_freq = 1.0 / (ROPE_THETA ** (np.arange(0, d, 2, dtype=np.float64) / d))
    pos = np.arange(s, dtype=np.float64)
    freqs = pos[:, None] * inv_freq[None, :]            # [s, d/2]
    emb = np.concatenate([freqs, freqs], axis=-1)       # [s, d]
    return np.cos(emb).astype(dtype), np.sin(emb).astype(dtype)


def _pmat(d):
    p = np.zeros((d, d), dtype=np.float32)
    h = d // 2
    for m in range(h):
        p[m + h, m] = -1.0
    for m in range(h, d):
        p[m - h, m] = 1.0
    return p


def make_in_maps(hidden_states, sequence_mask, Wqkv, Wo,
                 s=S, b=B, dmodel=D, nh_tot=N_HEADS, nh_loc=NH_LOC, d=DQK):
    bf = ml_dtypes.bfloat16
    cos, sin = _rope_tables(s, d)
    cosT = np.ascontiguousarray(cos.T).astype(bf)       # [d, s]
    sinT = np.ascontiguousarray(sin.T).astype(bf)
    pm = _pmat(d).astype(bf)
    ones = np.ones((128, 128), dtype=bf)
    scale = 1.0 / np.sqrt(np.float32(d))

    in_maps = []
    cores_per_batch = N_CORES // b
    for c in range(N_CORES):
        bi = c // cores_per_batch
        g = c % cores_per_batch
        h0 = g * nh_loc
        hsl = slice(h0 * d, (h0 + nh_loc) * d)
        nk, sc, nch = dmodel // 128, 512, s // 512
        xb = hidden_states[:, bi, :]                    # [s, dmodel]
        # [nch, 128, nk, sc]: xTq[ch, p, k, j] = x[ch*sc+j, k*128+p]
        xTq = np.ascontiguousarray(
            xb.T.reshape(nk, 128, nch, sc).transpose(2, 1, 0, 3)).astype(bf)
        hd = nh_loc * d

        def swz_w(w):   # [dmodel, hd] -> [128, nk, hd]
            return np.ascontiguousarray(
                w.reshape(nk, 128, hd).transpose(1, 0, 2)).astype(bf)

        wq = swz_w(Wqkv[:, 0 * nh_tot * d:1 * nh_tot * d][:, hsl] * scale)
        wk = swz_w(Wqkv[:, 1 * nh_tot * d:2 * nh_tot * d][:, hsl])
        wv = swz_w(Wqkv[:, 2 * nh_tot * d:3 * nh_tot * d][:, hsl])
        # [128, nh_loc, dmodel]: wo[p, h, e] = Wo[h*128+p, e]
        wo = np.ascontiguousarray(
            Wo[hsl, :].reshape(nh_loc, 128, dmodel).transpose(1, 0, 2)
        ).astype(bf)
        bias = np.where(sequence_mask[bi] == 0, -1e30, 0.0).astype(np.float32)
        maskbT = np.ascontiguousarray(bias.reshape(s // 128, 128).T)  # [128, ns]
        in_maps.append({
            "xTq": xTq, "wq": wq, "wk": wk, "wv": wv, "wo": wo,
            "cosT": cosT, "sinT": sinT, "maskb": maskbT,
            "pmat": pm, "ones": ones,
        })
    return in_maps


def kernel(hidden_states, sequence_mask, Wqkv, Wo):
    global LAST_RESULT
    from concourse.bass_utils import run_bass_kernel_spmd

    hidden_states = np.asarray(hidden_states)
    sequence_mask = np.asarray(sequence_mask)
    Wqkv = np.asarray(Wqkv)
    Wo = np.asarray(Wo)

    nc = _get_nc()
    in_maps = make_in_maps(hidden_states, sequence_mask, Wqkv, Wo)
    res = run_bass_kernel_spmd(
        nc, in_maps, list(range(N_CORES)),
        trace=bool(int(os.environ.get("KERNEL_TRACE", "0"))),
    )
    LAST_RESULT = res

    out = np.empty((S, B, D), dtype=np.float32)
    cores_per_batch = N_CORES // B
    for bi in range(B):
        acc = None
        for g in range(cores_per_batch):
            part = res.results[bi * cores_per_batch + g]["outT"]  # [D, S] bf16
            part = np.asarray(part, dtype=np.float32)
            acc = part if acc is None else acc + part
        out[:, bi, :] = acc.T
    return out


# revision 24
# speedup vs baseline: 1.0115x; 1.0115x over previous
"""Trainium2 Bass kernel for 16-head RoPE self-attention (S=2048, B=2, D=2048).

Sharding: 8 cores = 2 batches x 4 head-groups (4 heads each). Each core
computes qkv projection for its batch/heads, full attention over its 4
heads, and a partial output projection (its 4-head slice of Wo rows).
Host sums the 4 partial outputs per batch. No cross-core collectives.

v2 changes over the 406us baseline:
  - startup: chunk-0 q/k projections run k-outer with 4 concurrent PSUM
    accumulation groups, so the first matmul needs only one (wq, x)
    k-tile pair (~0.5MB) instead of the full 4.2MB contraction; input
    DMAs are staged in ascending-k slices split across the sync AND
    scalar queues (the only two HWDGE-capable issuers).
  - phase 2: output-projection work is emitted in per-et units and
    interleaved into the NEXT stripe's attention windows, which are
    otherwise ScalarE(EXP)-gated; remaining units drain at the end.
  - softmax denominator: full DVE pairwise tree to a single tile, then
    one ones-matmul per 512-half (2 instead of 8 PE ops per head).
    PV is emitted before the denominator matmuls so the PE never waits
    on the DVE add chain.
  - outputs are written bf16 (halves output DMA; host accumulates the
    4 partials in f32), with one 2KB-run DMA per 128-row block/stripe.
"""

import os
import numpy as np
import ml_dtypes

S, B, D = 2048, 2, 2048
N_HEADS, DQK = 16, 128
ROPE_THETA = 500000.0
N_CORES = 8
CORES_PER_BATCH = 4
NH_LOC = N_HEADS // CORES_PER_BATCH  # 4 heads per core

LAST_RESULT = None  # BassKernelResults of the most recent run (for test.py)

_NC_CACHE = {}


def _build_nc(s, dmodel, nh_loc, d=DQK, sc=512):
    import concourse.tile as tile
    from concourse import bacc, mybir

    bf16 = mybir.dt.bfloat16
    f32 = mybir.dt.float32
    nk = dmodel // 128      # contraction tiles for the projections
    ns = s // 128           # sequence tiles (key side)
    nch = s // sc           # sequence chunks (query side / moving dim)
    hd = nh_loc * d         # local head-dim total (512)
    ne = dmodel // 128      # output-embedding tiles

    nc = bacc.Bacc("TRN2", target_bir_lowering=False, debug=False)
    xTq = nc.dram_tensor("xTq", [nch, 128, nk, sc], bf16, kind="ExternalInput")
    wq = nc.dram_tensor("wq", [128, nk, hd], bf16, kind="ExternalInput")
    wk = nc.dram_tensor("wk", [128, nk, hd], bf16, kind="ExternalInput")
    wv = nc.dram_tensor("wv", [128, nk, hd], bf16, kind="ExternalInput")
    wo = nc.dram_tensor("wo", [128, hd // 128, dmodel], bf16,
                        kind="ExternalInput")
    cosT = nc.dram_tensor("cosT", [d, s], bf16, kind="ExternalInput")
    sinT = nc.dram_tensor("sinT", [d, s], bf16, kind="ExternalInput")
    maskb = nc.dram_tensor("maskb", [128, ns], f32, kind="ExternalInput")
    pmat = nc.dram_tensor("pmat", [d, d], bf16, kind="ExternalInput")
    ones = nc.dram_tensor("ones", [128, 128], bf16, kind="ExternalInput")
    outT = nc.dram_tensor("outT", [dmodel, s], bf16, kind="ExternalOutput")

    outT_r = outT.rearrange("(e p) s -> p e s", p=128)

    with tile.TileContext(nc) as tc:
        with tc.tile_pool(name="const", bufs=1) as constp, \
             tc.tile_pool(name="store", bufs=1) as storep:
            qT_sb = storep.tile([128, nh_loc, s], bf16)
            kT_sb = storep.tile([128, nh_loc, s], bf16)
            v_sb = storep.tile([128, ns, hd], bf16)
            attn_sb = storep.tile([128, nh_loc, s], bf16)

            # ---- phase 1: qkv projection + rope -------------------------
            with tc.tile_pool(name="wqkv", bufs=1) as wp, \
                 tc.tile_pool(name="xch", bufs=2) as xp, \
                 tc.tile_pool(name="rope", bufs=3) as rp, \
                 tc.tile_pool(name="ps1", bufs=1, space="PSUM") as ps1, \
                 tc.tile_pool(name="ps1v", bufs=2, space="PSUM") as ps1v, \
                 tc.tile_pool(name="ps1r", bufs=2, space="PSUM") as ps1r:
                wq_sb = wp.tile([128, nk, hd], bf16, tag="wq")
                wk_sb = wp.tile([128, nk, hd], bf16, tag="wk")
                wv_sb = wp.tile([128, nk, hd], bf16, tag="wv")
                xc0 = xp.tile([128, nk, sc], bf16, tag="xc")
                # staged ascending-k loads. wq on the scalar queue, xc0 on
                # sync (the two HWDGE issuers) so issue costs parallelize.
                # Big trailing loads go on sync so ring-credit blocking
                # never delays the scalar queue, which must run the
                # raw/rot/v copies from ~14us on.
                # pmat first: it's tiny and the rope rot-matmul blocks the
                # in-order PE queue if it lands late.
                pmat_sb = constp.tile([128, d], bf16)
                nc.scalar.dma_start(pmat_sb[:], pmat[:])
                g = 0
                for gq in (1, 1, 2, 4, 4, 4):
                    gs = slice(g, g + gq)
                    nc.scalar.dma_start(wq_sb[:, gs, :], wq[:, gs, :])
                    nc.sync.dma_start(xc0[:, gs, :], xTq[0, :, gs, :])
                    g += gq
                cos_sb = constp.tile([128, s], bf16)
                nc.scalar.dma_start(cos_sb[:], cosT[:])
                sin_sb = constp.tile([128, s], bf16)
                nc.scalar.dma_start(sin_sb[:], sinT[:])
                for gs in (slice(0, 4), slice(4, 8), slice(8, 12),
                           slice(12, 16)):
                    nc.sync.dma_start(wk_sb[:, gs, :], wk[:, gs, :])
                for gs in (slice(0, 4), slice(4, 8), slice(8, 12),
                           slice(12, 16)):
                    nc.sync.dma_start(wv_sb[:, gs, :], wv[:, gs, :])
                maskb_sb = constp.tile([128, ns], f32)
                nc.sync.dma_start(maskb_sb[:], maskb[:])
                ones_sb = constp.tile([128, 128], bf16)
                nc.sync.dma_start(ones_sb[:], ones[:])
                wo_sb = constp.tile([128, nh_loc, dmodel], bf16)
                nc.sync.dma_start(wo_sb[:], wo[:])

                def rope_combine(dstT, h, acc, csl, gi):
                    # q/k rope: raw -> (pmat matmul) rot; out = raw*cos+rot*sin
                    raw = rp.tile([128, sc], bf16, tag="raw", name=f"raw{gi}")
                    nc.scalar.copy(raw[:], acc[:])
                    rot_ps = ps1r.tile([128, sc], f32, tag="rot",
                                       name=f"rotp{gi}")
                    nc.tensor.matmul(rot_ps[:], pmat_sb[:], raw[:],
                                     start=True, stop=True)
                    rot = rp.tile([128, sc], bf16, tag="rot", name=f"rot{gi}")
                    nc.scalar.copy(rot[:], rot_ps[:])
                    t1 = rp.tile([128, sc], bf16, tag="t1", name=f"t1_{gi}")
                    nc.vector.tensor_mul(t1[:], raw[:], cos_sb[:, csl])
                    t2 = rp.tile([128, sc], bf16, tag="t2", name=f"t2_{gi}")
                    nc.vector.tensor_mul(t2[:], rot[:], sin_sb[:, csl])
                    nc.vector.tensor_add(dstT[:, h, csl], t1[:], t2[:])

                # lag-1 rope: emit proj p's rope AFTER proj p+1's matmuls so
                # the rot matmul never heads the PE queue while its raw
                # copy (ScalarE) is still in flight.
                pend_rope = [None]

                def queue_rope(*args):
                    if pend_rope[0] is not None:
                        pend_rope[0]()
                    pend_rope[0] = lambda: rope_combine(*args)

                def flush_rope():
                    if pend_rope[0] is not None:
                        pend_rope[0]()
                        pend_rope[0] = None

                gi = 0
                for ch in range(nch):
                    csl = slice(ch * sc, (ch + 1) * sc)
                    if ch == 0:
                        xc = xc0
                        # consume in DMA-arrival order: all q heads first
                        # (wq lands before wk)
                        proj_order = [("q", h) for h in range(nh_loc)] + \
                                     [("k", h) for h in range(nh_loc)]
                    else:
                        xc = xp.tile([128, nk, sc], bf16, tag="xc")
                        nc.sync.dma_start(xc[:], xTq[ch])
                        proj_order = [(t, h) for h in range(nh_loc)
                                      for t in ("q", "k")]
                    for t, h in proj_order:
                        hsl = slice(h * d, (h + 1) * d)
                        w_sb, dstT = ((wq_sb, qT_sb) if t == "q"
                                      else (wk_sb, kT_sb))
                        acc = ps1.tile([128, sc], f32, tag=f"acc{gi % 4}",
                                       name=f"acc{t}{h}c{ch}")
                        for k in range(nk):
                            nc.tensor.matmul(
                                acc[:], w_sb[:, k, hsl], xc[:, k, :],
                                start=(k == 0), stop=(k == nk - 1),
                            )
                        queue_rope(dstT, h, acc, csl, gi)
                        gi += 1
                    for stl in range(sc // 128):
                        st = ch * (sc // 128) + stl
                        ssl = slice(stl * 128, (stl + 1) * 128)
                        accv = ps1v.tile([128, hd], f32, tag="accv")
                        for k in range(nk):
                            nc.tensor.matmul(
                                accv[:], xc[:, k, ssl], wv_sb[:, k, :],
                                start=(k == 0), stop=(k == nk - 1),
                            )
                        if stl == 0:
                            flush_rope()
                        nc.scalar.copy(v_sb[:, st, :], accv[:])

            # ---- phase 2: attention + output projection -----------------
            sw = 2 * sc
            nst = s // sw
            with tc.tile_pool(name="expp", bufs=2) as expp, \
                 tc.tile_pool(name="ph2", bufs=3) as ph2p, \
                 tc.tile_pool(name="qd", bufs=5) as qdp, \
                 tc.tile_pool(name="outp", bufs=4) as outp, \
                 tc.tile_pool(name="ps2s", bufs=2, space="PSUM") as ps2s, \
                 tc.tile_pool(name="ps2p", bufs=2, space="PSUM") as ps2p, \
                 tc.tile_pool(name="ps2o", bufs=2, space="PSUM") as ps2o:

                def oproj_unit(ic, et, tail):
                    def emit():
                        esl = slice(et * 128, (et + 1) * 128)
                        ot = outp.tile([128, sw], bf16, tag="ot",
                                       name=f"ot{ic}_{et}")
                        for half in range(2):
                            osl = slice(ic * sw + half * sc,
                                        ic * sw + (half + 1) * sc)
                            op_ps = ps2o.tile([128, sc], f32, tag="oproj",
                                              name=f"op{ic}_{et}_{half}")
                            for ht in range(nh_loc):
                                nc.tensor.matmul(
                                    op_ps[:], wo_sb[:, ht, esl],
                                    attn_sb[:, ht, osl],
                                    start=(ht == 0), stop=(ht == nh_loc - 1),
                                )
                            dst = ot[:, half * sc:(half + 1) * sc]
                            # in the attention windows ScalarE is saturated
                            # with EXPs; in the tail it's idle, so split.
                            if tail and half == 0:
                                nc.scalar.copy(dst, op_ps[:])
                            else:
                                nc.vector.tensor_copy(dst, op_ps[:])
                        nc.sync.dma_start(
                            outT_r[:, et, ic * sw:(ic + 1) * sw], ot[:])
                    return emit

                pending = []
                for ic in range(nst):
                    for h in range(nh_loc):
                        hsl = slice(h * d, (h + 1) * d)
                        ex = expp.tile([128, ns, sw], bf16, tag="exp",
                                       name=f"ex{ic}_{h}")
                        # Denominator as RUNNING partial sums: exps 0..11 go
                        # through a small tree while later jts stream; exps
                        # 12..15 are folded in one short add each, emitted
                        # right after their EXP, so the total is ready
                        # ~0.6us after the last EXP (a leaf-pairwise tree
                        # needs ~3.4us of post-EXP latency, which stalled
                        # the next head's scores and re-throttled HAM).
                        quads = []
                        run = None
                        for jt in range(ns):
                            jsl = slice(jt * 128, (jt + 1) * 128)
                            sc_ps = ps2s.tile([128, sw], f32, tag="scores",
                                              name=f"scps{ic}_{h}_{jt}")
                            for half in range(2):
                                qsl = slice(ic * sw + half * sc,
                                            ic * sw + (half + 1) * sc)
                                nc.tensor.matmul(
                                    sc_ps[:, half * sc:(half + 1) * sc],
                                    kT_sb[:, h, jsl], qT_sb[:, h, qsl],
                                    start=True, stop=True)
                            nc.scalar.activation(
                                ex[:, jt, :], sc_ps[:],
                                mybir.ActivationFunctionType.Exp,
                                bias=maskb_sb[:, jt:jt + 1], scale=1.0,
                            )
                            if jt in (3, 7, 11):
                                a = jt // 4
                                p0 = ph2p.tile([128, sw], bf16, tag="pair")
                                nc.vector.tensor_add(p0[:], ex[:, 4 * a, :],
                                                     ex[:, 4 * a + 1, :])
                                p1 = ph2p.tile([128, sw], bf16, tag="pair2")
                                nc.vector.tensor_add(
                                    p1[:], ex[:, 4 * a + 2, :],
                                    ex[:, 4 * a + 3, :])
                                q4 = qdp.tile([128, sw], bf16, tag="quad")
                                nc.vector.tensor_add(q4[:], p0[:], p1[:])
                                quads.append(q4)
                                if a == 2:
                                    s01 = qdp.tile([128, sw], bf16,
                                                   tag="s01", bufs=2)
                                    nc.vector.tensor_add(s01[:], quads[0][:],
                                                         quads[1][:])
                                    run = qdp.tile([128, sw], bf16,
                                                   tag="run", bufs=2)
                                    nc.vector.tensor_add(run[:], s01[:],
                                                         quads[2][:])
                            elif jt >= 12:
                                nxt = qdp.tile([128, sw], bf16, tag="run",
                                               bufs=2,
                                               name=f"run{ic}_{h}_{jt}")
                                nc.vector.tensor_add(nxt[:], run[:],
                                                     ex[:, jt, :])
                                run = nxt
                            # fill the EXP-gated stream with oproj work from
                            # the previous stripe
                            if pending and jt in (3, 7, 11, 15):
                                pending.pop(0)()
                        # r lives in the oproj pool (idle in stripe 0,
                        # compatible rotation in stripe 1) so the scores
                        # rotation is never held by the r->recip chain —
                        # that stalled every next head's scores 2.6-4.9us
                        # and re-throttled HAM.
                        inv = ph2p.tile([128, sw], f32, tag="inv")
                        for half in range(2):
                            fsl = slice(half * sc, (half + 1) * sc)
                            r_ps = ps2o.tile([128, sc], f32, tag="oproj",
                                             name=f"rps{ic}_{h}_{half}")
                            nc.tensor.matmul(r_ps[:], ones_sb[:],
                                             run[:, fsl],
                                             start=True, stop=True)
                            nc.vector.reciprocal_approx_fast(inv[:, fsl],
                                                             r_ps[:])
                        for half in range(2):
                            fsl = slice(half * sc, (half + 1) * sc)
                            asl = slice(ic * sw + half * sc,
                                        ic * sw + (half + 1) * sc)
                            pv_ps = ps2p.tile([128, sc], f32, tag="pv")
                            for jt in range(ns):
                                nc.tensor.matmul(
                                    pv_ps[:], v_sb[:, jt, hsl],
                                    ex[:, jt, fsl],
                                    start=(jt == 0), stop=(jt == ns - 1),
                                )
                            nc.vector.tensor_mul(attn_sb[:, h, asl],
                                                 pv_ps[:], inv[:, fsl])
                    if ic == 0:
                        pending = [oproj_unit(0, et, False)
                                   for et in range(ne)]
                for f in pending:
                    f()

                def oproj_half(ic, et, half):
                    esl = slice(et * 128, (et + 1) * 128)
                    osl = slice(ic * sw + half * sc,
                                ic * sw + (half + 1) * sc)
                    op_ps = ps2o.tile([128, sc], f32, tag="oproj",
                                      name=f"oph{ic}_{et}_{half}")
                    for ht in range(nh_loc):
                        nc.tensor.matmul(
                            op_ps[:], wo_sb[:, ht, esl],
                            attn_sb[:, ht, osl],
                            start=(ht == 0), stop=(ht == nh_loc - 1),
                        )
                    oth = outp.tile([128, sc], bf16, tag="oth",
                                    name=f"oth{ic}_{et}_{half}")
                    if et % 2 == 0:
                        nc.scalar.copy(oth[:], op_ps[:])
                    else:
                        nc.vector.tensor_copy(oth[:], op_ps[:])
                    nc.sync.dma_start(outT_r[:, et, osl], oth[:])

                for half in range(2):
                    for et in range(ne):
                        oproj_half(nst - 1, et, half)

    nc.compile()
    return nc


def _get_nc(s=S, dmodel=D, nh_loc=NH_LOC):
    key = (s, dmodel, nh_loc)
    if key not in _NC_CACHE:
        _NC_CACHE[key] = _build_nc(s, dmodel, nh_loc)
    return _NC_CACHE[key]


def _rope_tables(s, d, dtype=np.float32):
    inv_freq = 1.0 / (ROPE_THETA ** (np.arange(0, d, 2, dtype=np.float64) / d))
    pos = np.arange(s, dtype=np.float64)
    freqs = pos[:, None] * inv_freq[None, :]            # [s, d/2]
    emb = np.concatenate([freqs, freqs], axis=-1)       # [s, d]
    return np.cos(emb).astype(dtype), np.sin(emb).astype(dtype)


def _pmat(d):
    p = np.zeros((d, d), dtype=np.float32)
    h = d // 2
    for m in range(h):
        p[m + h, m] = -1.0
    for m in range(h, d):
        p[m - h, m] = 1.0
    return p


def make_in_maps(hidden_states, sequence_mask, Wqkv, Wo,
                 s=S, b=B, dmodel=D, nh_tot=N_HEADS, nh_loc=NH_LOC, d=DQK):
    bf = ml_dtypes.bfloat16
    cos, sin = _rope_tables(s, d)
    cosT = np.ascontiguousarray(cos.T).astype(bf)       # [d, s]
    sinT = np.ascontiguousarray(sin.T).astype(bf)
    pm = _pmat(d).astype(bf)
    ones = np.ones((128, 128), dtype=bf)
    scale = 1.0 / np.sqrt(np.float32(d))

    in_maps = []
    cores_per_batch = N_CORES // b
    for c in range(N_CORES):
        bi = c // cores_per_batch
        g = c % cores_per_batch
        h0 = g * nh_loc
        hsl = slice(h0 * d, (h0 + nh_loc) * d)
        nk, sc, nch = dmodel // 128, 512, s // 512
        xb = hidden_states[:, bi, :]                    # [s, dmodel]
        # [nch, 128, nk, sc]: xTq[ch, p, k, j] = x[ch*sc+j, k*128+p]
        xTq = np.ascontiguousarray(
            xb.T.reshape(nk, 128, nch, sc).transpose(2, 1, 0, 3)).astype(bf)
        hd = nh_loc * d

        def swz_w(w):   # [dmodel, hd] -> [128, nk, hd]
            return np.ascontiguousarray(
                w.reshape(nk, 128, hd).transpose(1, 0, 2)).astype(bf)

        wq = swz_w(Wqkv[:, 0 * nh_tot * d:1 * nh_tot * d][:, hsl] * scale)
        wk = swz_w(Wqkv[:, 1 * nh_tot * d:2 * nh_tot * d][:, hsl])
        wv = swz_w(Wqkv[:, 2 * nh_tot * d:3 * nh_tot * d][:, hsl])
        # [128, nh_loc, dmodel]: wo[p, h, e] = Wo[h*128+p, e]
        wo = np.ascontiguousarray(
            Wo[hsl, :].reshape(nh_loc, 128, dmodel).transpose(1, 0, 2)
        ).astype(bf)
        bias = np.where(sequence_mask[bi] == 0, -1e30, 0.0).astype(np.float32)
        maskbT = np.ascontiguousarray(bias.reshape(s // 128, 128).T)  # [128, ns]
        in_maps.append({
            "xTq": xTq, "wq": wq, "wk": wk, "wv": wv, "wo": wo,
            "cosT": cosT, "sinT": sinT, "maskb": maskbT,
            "pmat": pm, "ones": ones,
        })
    return in_maps


def kernel(hidden_states, sequence_mask, Wqkv, Wo):
    global LAST_RESULT
    from concourse.bass_utils import run_bass_kernel_spmd

    hidden_states = np.asarray(hidden_states)
    sequence_mask = np.asarray(sequence_mask)
    Wqkv = np.asarray(Wqkv)
    Wo = np.asarray(Wo)

    nc = _get_nc()
    in_maps = make_in_maps(hidden_states, sequence_mask, Wqkv, Wo)
    res = run_bass_kernel_spmd(
        nc, in_maps, list(range(N_CORES)),
        trace=bool(int(os.environ.get("KERNEL_TRACE", "0"))),
    )
    LAST_RESULT = res

    out = np.empty((S, B, D), dtype=np.float32)
    cores_per_batch = N_CORES // B
    for bi in range(B):
        acc = None
        for g in range(cores_per_batch):
            part = res.results[bi * cores_per_batch + g]["outT"]  # [D, S] bf16
            part = np.asarray(part, dtype=np.float32)
            acc = part if acc is None else acc + part
        out[:, bi, :] = acc.T
    return out
